# revision 37
# baseline (speedup 1.0000x reference)
"""AggregationMPNN Trainium2 kernel (data-parallel over the graph/batch dim).

Math (per graph, matching the reference):
  hidden = zeropad(nodes)                                [V, H]
  3x message pass:
    att_p[w,e,m] = hidden[w] @ att_W[e]; msg_p likewise  (biases are zero)
    Because edges[v,w,:] is one-hot (masked), softmax attention collapses to
      numer[v,m] = sum_{w,e} edges[v,w,e] * exp(att_p[w,e,m]) * msg_p[w,e,m]
      denom[v,m] = sum_{w,e} edges[v,w,e] * exp(att_p[w,e,m])
      message    = numer / (denom + 1e-30)
    GRU update, applied only where node degree > 0 (denom > 0).
  readout: sum_v sigmoid([h,nodes]@Wa) * (h@We) * mask

Layout: 8 graphs/core => 512 node slots. Hidden is kept TRANSPOSED in SBUF
(hT[H=256, 512]) feeding projections as lhsT and the GRU as rhs. All matmul
operands are bf16 (fast FWL weight loads, halved input DMA); PSUM accumulation
stays fp32. sigmoid(x) is computed as 0.5*tanh(0.5x)+0.5 so every activation
uses the exp_and_others table set (one ACT_TABLE_LOAD total); the 0.5/0.5
affine fixups are folded away: Wnh is pre-halved on the host so
r*gh_n = (tanh+1)*gh_n', the node mask is stored as 0.5/0 and the matching 2x
lands in the readout gate (tanh+1). The per-pair edge gather is one matmul per
edge type over a block-diagonal [128,128] edge tile (both graphs of a pair
packed into the contraction dim); the denominator eps rides in as a constant
rank-1 matmul. Emission order is tuned so the PE never queues behind the
per-half GRU tail chain (gather of ready pairs first at pass starts, readout
emitted after the second half's matmuls in the last pass).
"""

import sys

sys.path.insert(0, "/opt/trn_rl_repo")

import numpy as np

N, V, E, NF, H, M = 64, 64, 8, 64, 256, 128
OUT = H
NCORES = 8
G = N // NCORES          # graphs per core
VG = V * G               # node slots per core (512)
NPAIR = G // 2           # graph pairs per core (4)
EPS = 1e-30
MASK_THRESH = 1e-20      # denom > thresh <=> node has a neighbour (real
                         # denoms are >= exp(min att) >> 1e-20; eps = 1e-30)
HCOL = 256               # node columns per pipeline half (2 graph pairs)

_BUILT = None            # cached compiled bass module
TRACE = False            # test.py sets kernel.TRACE = True for profiling
LAST_RESULTS = None      # BassKernelResults of the last run (for profiling)


def _emit(ctx, tc, d, npasses=3, dbg=False):
    import concourse.bass as bass  # noqa: F401
    from concourse import mybir
    from concourse.masks import make_identity

    nc = tc.nc
    FP = mybir.dt.float32
    BF = mybir.dt.bfloat16
    AF = mybir.ActivationFunctionType
    OP = mybir.AluOpType
    AX = mybir.AxisListType

    def mm(out, lhsT, rhs, start, stop):
        nc.tensor.matmul(out, lhsT, rhs, start=start, stop=stop)

    consts = ctx.enter_context(tc.tile_pool(name="consts", bufs=1))
    work = ctx.enter_context(tc.tile_pool(name="work", bufs=3))
    pp_ps = ctx.enter_context(tc.tile_pool(name="pp_ps", bufs=3, space="PSUM"))
    gat_ps = ctx.enter_context(tc.tile_pool(name="gat_ps", bufs=2, space="PSUM"))
    gru_ps = ctx.enter_context(tc.tile_pool(name="gru_ps", bufs=3, space="PSUM"))

    # ---- persistent SBUF state ----
    hT0 = consts.tile([128, 2, HCOL], BF)       # hidden^T, node cols 0:256
    hT1 = consts.tile([128, 2, HCOL], BF)       # hidden^T, node cols 256:512
    hTh = (hT0, hT1)
    nodesT = consts.tile([64, VG], BF)          # nodes^T
    wc = consts.tile([128, 2, 2 * E * M], BF)   # [att | msg] proj weights
    edge = consts.tile([128, NPAIR, E, 128], BF)  # block-diag edges^T per pair
    wrz = consts.tile([128, 3, 2 * H], BF)      # GRU r,z weights (K=[h;m])
    wnh = consts.tile([128, 2, H], BF)          # GRU n gate, hidden (x0.5)
    wni = consts.tile([128, H], BF)             # GRU n gate, message part
    wga = consts.tile([128, 3, OUT], BF)        # readout gate weights
    wge = consts.tile([128, 2, OUT], BF)        # readout emb weights
    identB = consts.tile([128, 128], BF)
    epsrow = consts.tile([128, 128], BF)        # row 0 = EPS, rest 0
    ones2 = consts.tile([128, 2, M], BF)        # all-ones rhs for the eps mm
    AB = consts.tile([128, NPAIR, E * 2 * M], BF)   # per e: [A(128) | B(128)]
    msgT0 = consts.tile([128, HCOL], BF)
    msgT1 = consts.tile([128, HCOL], BF)
    msgTh = (msgT0, msgT1)
    maskh = consts.tile([128, 2, VG], BF)       # 0.5*mask, bcast over parts
    red = consts.tile([128, 2, G], FP)

    # ---- input DMAs, ordered by when the consuming phase needs them ----
    nc.sync.dma_start(out=nodesT[:], in_=d["nodesT"][:])
    # pass-0 projections contract only hidden rows 0:64 (= node features);
    # the very first matmul consumes only the q=0 slice
    nc.sync.dma_start(out=wc[0:64, 0, 0:512], in_=d["Wc"][0:64, 0:512])
    nc.sync.dma_start(out=wc[0:64, 0, 512:2048], in_=d["Wc"][0:64, 512:2048])
    # the small pass-0 GRU weight DMAs cut ahead of the edge bulk: startup
    # HBM bandwidth is saturated and the GRU (~11us in) stalled on these,
    # while the gathers (~7us) have slack
    for k in (0, 2):
        nc.sync.dma_start(out=wrz[:, k, :],
                          in_=d["Wrz"][k * 128:(k + 1) * 128, :])
    nc.sync.dma_start(out=wni[:], in_=d["Wni"][:])
    nc.sync.dma_start(out=wnh[:, 0, :], in_=d["Wnh"][0:128, :])
    for c in range(NPAIR):                   # pass-0 gathers
        nc.sync.dma_start(out=edge[:, c, :, :], in_=d["edges_p"][c])
    make_identity(nc, identB[:])
    nc.gpsimd.memset(epsrow[:], 0.0)
    nc.gpsimd.memset(epsrow[0:1, :], EPS)
    nc.gpsimd.memset(ones2[:], 1.0)
    # init hidden^T = [nodes^T ; 0] (on GpSimd: DVE stays free for pass 0)
    for i in range(2):
        nc.gpsimd.memset(hTh[i][:], 0.0)
        nc.gpsimd.tensor_copy(out=hTh[i][0:64, 0, :],
                              in_=nodesT[:, i * HCOL:(i + 1) * HCOL])
    # needed from pass 1 on
    nc.sync.dma_start(out=wrz[:, 1, :], in_=d["Wrz"][128:256, :])
    nc.sync.dma_start(out=wnh[:, 1, :], in_=d["Wnh"][128:256, :])
    nc.sync.dma_start(out=wc[64:128, 0, :], in_=d["Wc"][64:128, :])
    nc.sync.dma_start(out=wc[:, 1, :], in_=d["Wc"][128:256, :])
    # readout weights, needed only in the last pass
    nc.sync.dma_start(out=wga[:, 0:2, :],
                      in_=d["Wga"][0:256, :].rearrange("(k p) c -> p k c",
                                                       p=128))
    nc.sync.dma_start(out=wga[0:64, 2, :], in_=d["Wga"][256:320, :])
    nc.sync.dma_start(out=wge[:],
                      in_=d["Wge"][:].rearrange("(k p) c -> p k c", p=128))

    def keep_warm(n):
        # throwaway matmuls that slot into an upcoming PE-idle window (FIFO
        # order): they hold the HAM activity window open so the clock stays
        # at 8/8 across the dependency stall, and cost nothing while idle.
        # Allocated from the pp pool -- its buffers are idle at the pass
        # boundaries where these run (the gru pool's are still held by the
        # in-flight GRU banks, which would stall the dummies themselves).
        kw = pp_ps.tile([128, 256], FP, tag="pp")
        for i in range(n):
            mm(kw[:], identB[:], ones2[:], i == 0, i == n - 1)

    def emit_proj(cs, pass0=False):
        # projections + A/B construction, one PSUM bank per (half, cc)
        for c in cs:
            abv = AB[:, c, :].rearrange("p (e x) -> p e x", x=2 * M)
            for half in range(2):        # 0: att (exp->B) | 1: msg (*B->A)
                for cc in range(2):
                    q = half * 2 + cc
                    pp = pp_ps.tile([128, 512], FP, tag="pp")
                    if pass0:
                        mm(pp[:], nodesT[:, c * 128:(c + 1) * 128],
                           wc[0:64, 0, q * 512:(q + 1) * 512], True, True)
                    else:
                        for k in range(2):
                            lh = hTh[c // 2][:, k,
                                             (c % 2) * 128:(c % 2 + 1) * 128]
                            mm(pp[:], lh, wc[:, k, q * 512:(q + 1) * 512],
                               k == 0, k == 1)
                    ppv = pp[:].rearrange("p (e m) -> p e m", m=M)
                    esl = slice(cc * 4, (cc + 1) * 4)
                    if half == 0:
                        nc.scalar.activation(out=abv[:, esl, M:2 * M],
                                             in_=ppv, func=AF.Exp)
                    else:
                        nc.vector.tensor_mul(out=abv[:, esl, 0:M], in0=ppv,
                                             in1=abv[:, esl, M:2 * M])

    def emit_gather(hf, first):
        # one matmul per (pair, edge type) + a rank-1 eps matmul
        gat = gat_ps.tile([128, 2, 2, M], FP, tag="gat")
        for ci in range(2):
            c = 2 * hf + ci
            mm(gat[:, ci, :, :], epsrow[:], ones2[:], True, False)
            for e in range(E):
                mm(gat[:, ci, :, :], edge[:, c, e, :],
                   AB[:, c, e * 2 * M:(e + 1) * 2 * M], False, e == E - 1)
        rec = work.tile([128, 2, M], FP, tag="rec")
        nc.vector.reciprocal_approx_fast(out=rec[:], in_=gat[:, :, 1, :])
        msgN = work.tile([128, 2, M], BF, tag="msgN")
        nc.vector.tensor_mul(out=msgN[:], in0=gat[:, :, 0, :], in1=rec[:])
        den_sb = None
        if first:
            den_sb = work.tile([128, 2, M], BF, tag="den")
            nc.vector.tensor_scalar(den_sb[:], gat[:, :, 1, :], MASK_THRESH,
                                    0.5, OP.is_gt, OP.mult)
        return msgN, den_sb

    def emit_msgT(hf, msgN, den_sb):
        sl = slice(hf * HCOL, (hf + 1) * HCOL)
        mt = gru_ps.tile([128, HCOL], BF, tag="g")
        for ci in range(2):
            nc.tensor.transpose(mt[:, ci * 128:(ci + 1) * 128],
                                msgN[:, ci, :], identB[:])
        nc.scalar.activation(out=msgTh[hf][:], in_=mt[:], func=AF.Copy)
        if den_sb is not None:
            dt = gru_ps.tile([128, HCOL], BF, tag="g")
            for ci in range(2):
                nc.tensor.transpose(dt[:, ci * 128:(ci + 1) * 128],
                                    den_sb[:, ci, :], identB[:])
            nc.scalar.activation(out=maskh[:, 0, sl], in_=dt[:], func=AF.Copy)
            nc.scalar.activation(out=maskh[:, 1, sl], in_=maskh[:, 0, sl],
                                 func=AF.Copy)

    def emit_gru(hf, first, final=False):
        # final=True: the caller consumes u2 via the incremental readout and
        # nothing reads hT afterwards, so the in-place hT update is skipped
        # (and the readout-base matmuls are emitted between the GRU matmuls
        # and the tail chain to keep the PE fed)
        sl = slice(hf * HCOL, (hf + 1) * HCOL)
        ks = (0, 2) if first else (0, 1, 2)
        rhs_for = {0: hTh[hf][:, 0, :], 1: hTh[hf][:, 1, :], 2: msgTh[hf][:]}
        ps_r = gru_ps.tile([128, 2, HCOL], FP, tag="g")
        for jj in range(2):
            for i, k in enumerate(ks):
                mm(ps_r[:, jj, :], wrz[:, k, jj * 128:(jj + 1) * 128],
                   rhs_for[k], i == 0, i == len(ks) - 1)
        ps_z = gru_ps.tile([128, 2, HCOL], FP, tag="g")
        for jj in range(2):
            for i, k in enumerate(ks):
                mm(ps_z[:, jj, :],
                   wrz[:, k, 256 + jj * 128:256 + (jj + 1) * 128],
                   rhs_for[k], i == 0, i == len(ks) - 1)
        gin = gru_ps.tile([128, 2, HCOL], FP, tag="g")
        for jj in range(2):
            mm(gin[:, jj, :], wni[:, jj * 128:(jj + 1) * 128],
               msgTh[hf][:], True, True)
        ghn = gru_ps.tile([128, 2, HCOL], FP, tag="g")
        hks = (0,) if first else (0, 1)
        for jj in range(2):
            for i, k in enumerate(hks):
                mm(ghn[:, jj, :], wnh[:, k, jj * 128:(jj + 1) * 128],
                   hTh[hf][:, k, :], i == 0, i == len(hks) - 1)
        base = None
        if final:
            base = emit_readout_base(hf)
            keep_warm(8)
        # r = sigmoid(x) = 0.5*(tanh(0.5x)+1); Wnh is pre-halved so
        # r*gh_n = (tanh(0.5x)+1) * ghn'
        rt = work.tile([128, 2, HCOL], BF, tag="rt")
        nc.scalar.activation(out=rt[:], in_=ps_r[:], func=AF.Tanh, scale=0.5)
        zt = work.tile([128, 2, HCOL], BF, tag="zt")
        nc.scalar.activation(out=zt[:], in_=ps_z[:], func=AF.Tanh, scale=-0.5)
        # mz = mask*(1-z) = (tanh(-0.5x)+1) * maskh
        # (two ops: Pool has no scalar_tensor_tensor opcode on hardware, and
        # the single-scalar ADD,BYPASS form hits an 8.9us ucode path that
        # also starves concurrent DVE SBUF reads -- use MULTIPLY,ADD)
        zc1 = work.tile([128, 2, HCOL], BF, tag="zc1")
        nc.gpsimd.tensor_scalar(zc1[:], zt[:], 1.0, 1.0, OP.mult, OP.add)
        mz = work.tile([128, 2, HCOL], BF, tag="mz")
        nc.gpsimd.tensor_mul(out=mz[:], in0=zc1[:], in1=maskh[:, :, sl])
        t1 = work.tile([128, 2, HCOL], BF, tag="t1")
        t2 = work.tile([128, 2, HCOL], BF, tag="t2")
        nT = work.tile([128, 2, HCOL], BF, tag="nT")
        dd = work.tile([128, 2, HCOL], BF, tag="dd")
        u2 = work.tile([128, 2, HCOL], BF, tag="u2")
        if not final:
            nc.vector.scalar_tensor_tensor(out=t1[:], in0=rt[:], scalar=1.0,
                                           in1=ghn[:], op0=OP.add, op1=OP.mult)
            nc.vector.tensor_add(out=t2[:], in0=gin[:], in1=t1[:])
            nc.scalar.activation(out=nT[:], in_=t2[:], func=AF.Tanh)
            # h' = h + mz*(n - h)
            nc.vector.tensor_tensor(out=dd[:], in0=nT[:], in1=hTh[hf][:],
                                    op=OP.subtract)
            nc.vector.tensor_mul(out=u2[:], in0=mz[:], in1=dd[:])
            nc.vector.tensor_add(out=hTh[hf][:], in0=hTh[hf][:], in1=u2[:])
            return
        # final half: run the tail per H-chunk so the readout-delta matmuls
        # of chunk 0 keep the PE (and its clock) busy during chunk 1's chain
        gps, eps2 = base
        for k in range(2):
            ck = slice(k, k + 1)
            nc.vector.scalar_tensor_tensor(out=t1[:, ck, :], in0=rt[:, ck, :],
                                           scalar=1.0, in1=ghn[:, ck, :],
                                           op0=OP.add, op1=OP.mult)
            nc.vector.tensor_add(out=t2[:, ck, :], in0=gin[:, ck, :],
                                 in1=t1[:, ck, :])
            nc.scalar.activation(out=nT[:, ck, :], in_=t2[:, ck, :],
                                 func=AF.Tanh)
        for k in range(2):
            ck = slice(k, k + 1)
            nc.vector.tensor_tensor(out=dd[:, ck, :], in0=nT[:, ck, :],
                                    in1=hTh[hf][:, ck, :], op=OP.subtract)
            nc.vector.tensor_mul(out=u2[:, ck, :], in0=mz[:, ck, :],
                                 in1=dd[:, ck, :])
            for jj in range(2):
                mm(gps[:, jj, :], wga[:, k, jj * 128:(jj + 1) * 128],
                   u2[:, k, :], False, k == 1 and jj == 1)
            for jj in range(2):
                mm(eps2[:, jj, :], wge[:, k, jj * 128:(jj + 1) * 128],
                   u2[:, k, :], False, k == 1 and jj == 1)
        emit_readout_tail(hf, gps, eps2, split=True)

    def emit_readout_base(hf, stop=False):
        # gate/emb projections of the PRE-update hidden state (and nodes);
        # linear in h, so the h' = h + u2 correction can accumulate later
        # ONE accumulation group per PSUM bank (start lazily zeroes the whole
        # 2KB zero region; both jj column blocks accumulate inside it)
        sl = slice(hf * HCOL, (hf + 1) * HCOL)
        gps = pp_ps.tile([128, 2, HCOL], FP, tag="pp")
        for n, jj in enumerate(range(2)):
            mm(gps[:, jj, :], wga[:, 0, jj * 128:(jj + 1) * 128],
               hTh[hf][:, 0, :], n == 0, False)
            mm(gps[:, jj, :], wga[:, 1, jj * 128:(jj + 1) * 128],
               hTh[hf][:, 1, :], False, False)
            mm(gps[:, jj, :], wga[0:64, 2, jj * 128:(jj + 1) * 128],
               nodesT[:, sl], False, stop and n == 1)
        eps2 = pp_ps.tile([128, 2, HCOL], FP, tag="pp")
        for n, jj in enumerate(range(2)):
            mm(eps2[:, jj, :], wge[:, 0, jj * 128:(jj + 1) * 128],
               hTh[hf][:, 0, :], n == 0, False)
            mm(eps2[:, jj, :], wge[:, 1, jj * 128:(jj + 1) * 128],
               hTh[hf][:, 1, :], False, stop and n == 1)
        return gps, eps2

    def emit_readout_tail(hf, gps, eps2, split=False):
        sl = slice(hf * HCOL, (hf + 1) * HCOL)
        # 2*gate = tanh(0.5x)+1 pairs with the 0.5-scaled mask; split=True
        # pipelines the chain per jj chunk (kernel-final critical path)
        gt = work.tile([128, 2, HCOL], BF, tag="rt")
        tt = work.tile([128, 2, HCOL], BF, tag="t1")
        t2r = work.tile([128, 2, HCOL], BF, tag="t2")
        jjs = (slice(0, 1), slice(1, 2)) if split else (slice(0, 2),)
        for cj in jjs:
            nc.scalar.activation(out=gt[:, cj, :], in_=gps[:, cj, :],
                                 func=AF.Tanh, scale=0.5)
            nc.vector.scalar_tensor_tensor(out=tt[:, cj, :], in0=gt[:, cj, :],
                                           scalar=1.0, in1=eps2[:, cj, :],
                                           op0=OP.add, op1=OP.mult)
            nc.vector.tensor_mul(out=t2r[:, cj, :], in0=tt[:, cj, :],
                                 in1=maskh[:, cj, sl])
            nc.vector.tensor_reduce(
                out=red[:, cj, hf * 4:(hf + 1) * 4],
                in_=t2r[:, cj, :].rearrange("p j (g v) -> p j g v", v=V),
                axis=AX.X, op=OP.add)
        # ship this half's sums immediately; the host does the tiny
        # [d,j,g] -> [g, j*128+d] transpose (saves the on-device PE
        # transpose + copy and overlaps the DMA with the other half)
        nc.sync.dma_start(out=d["red"][:, :, hf * 4:(hf + 1) * 4],
                          in_=red[:, :, hf * 4:(hf + 1) * 4])

    def emit_readout(hf):
        gps, eps2 = emit_readout_base(hf, stop=True)
        emit_readout_tail(hf, gps, eps2)



    for p in range(npasses):
        first = p == 0
        last = p == npasses - 1
        if first:
            emit_proj((0, 1, 2, 3), pass0=True)
            msgN0, den0 = emit_gather(0, first)
        else:
            # gather of pairs 0,1 is ready (their AB was finished last pass):
            # emit it before the projections so the PE does not queue behind
            # the previous half's GRU tail waiting for the new hT
            msgN0, den0 = emit_gather(0, first)
            keep_warm(8)
            emit_proj((2, 3))
        emit_msgT(0, msgN0, den0)
        emit_gru(0, first)
        msgN1, den1 = emit_gather(1, first)
        if not last:
            # next pass's first two projection pairs: hT half A is final,
            # and the PE would otherwise idle behind this half's GRU chain
            emit_proj((0, 1))
        else:
            # half 0's readout is ready now; emitting it before half 1's
            # GRU keeps its DVE ops out of the half-1 critical tail chain
            emit_readout(0)
        emit_msgT(1, msgN1, den1)
        emit_gru(1, first, final=last)

        if dbg:
            nc.sync.dma_start(out=d[f"dbg_hT{p}"][:, :, 0:HCOL],
                              in_=hTh[0][:])
            nc.sync.dma_start(out=d[f"dbg_hT{p}"][:, :, HCOL:VG],
                              in_=hTh[1][:])
            if p == 0:
                nc.sync.dma_start(out=d["dbg_AB"][:], in_=AB[:])
                nc.sync.dma_start(out=d["dbg_msgT"][:, 0:HCOL],
                                  in_=msgTh[0][:])
                nc.sync.dma_start(out=d["dbg_msgT"][:, HCOL:VG],
                                  in_=msgTh[1][:])
                nc.sync.dma_start(out=d["dbg_maskb"][:], in_=maskh[:, 0, :])




def build(npasses=3, dbg=False):
    """Build + compile the bass module (cached)."""
    global _BUILT
    if _BUILT is not None and not dbg and npasses == 3:
        return _BUILT
    import concourse.bacc as bacc
    import concourse.tile as tile
    from concourse import mybir

    FP = mybir.dt.float32
    BF = mybir.dt.bfloat16
    nc = bacc.Bacc("TRN2", target_bir_lowering=False)
    d = {
        "nodesT": nc.dram_tensor("nodesT", [NF, VG], BF, kind="ExternalInput"),
        "edges_p": nc.dram_tensor("edges_p", [NPAIR, 128, E, 128], BF,
                                  kind="ExternalInput"),
        "Wc": nc.dram_tensor("Wc", [H, 2 * E * M], BF, kind="ExternalInput"),
        "Wrz": nc.dram_tensor("Wrz", [H + M, 2 * H], BF, kind="ExternalInput"),
        "Wnh": nc.dram_tensor("Wnh", [H, H], BF, kind="ExternalInput"),
        "Wni": nc.dram_tensor("Wni", [M, H], BF, kind="ExternalInput"),
        "Wga": nc.dram_tensor("Wga", [H + NF, OUT], BF, kind="ExternalInput"),
        "Wge": nc.dram_tensor("Wge", [H, OUT], BF, kind="ExternalInput"),
        "red": nc.dram_tensor("red", [128, 2, G], FP, kind="ExternalOutput"),
    }
    if dbg:
        for name, shape in [
            ("dbg_AB", [128, NPAIR, E * 2 * M]),
            ("dbg_msgT", [128, VG]),
            ("dbg_maskb", [128, VG]),
        ] + [(f"dbg_hT{p}", [128, 2, VG]) for p in range(npasses)]:
            d[name] = nc.dram_tensor(name, shape, BF, kind="ExternalOutput")
    from contextlib import ExitStack

    with tile.TileContext(nc) as tc:
        with ExitStack() as ctx:
            _emit(ctx, tc, d, npasses=npasses, dbg=dbg)
    nc.compile()
    if not dbg and npasses == 3:
        _BUILT = nc
    return nc


def make_in_maps(nodes, edges, msg_W, msg_b, att_W, att_b, gru_W_ih, gru_W_hh,
                 gru_b_ih, gru_b_hh, gather_att_W, gather_att_b, gather_emb_W,
                 gather_emb_b):
    """Host-side layout prep (transposes/concats/bf16 cast) + sharding."""
    import ml_dtypes

    bf = ml_dtypes.bfloat16
    for b in (msg_b, att_b, gru_b_ih, gru_b_hh, gather_att_b, gather_emb_b):
        if np.abs(np.asarray(b)).max() > 0:
            raise NotImplementedError("nonzero biases not folded on device")
    wc = np.concatenate([
        np.ascontiguousarray(att_W.transpose(1, 0, 2)).reshape(H, E * M),
        np.ascontiguousarray(msg_W.transpose(1, 0, 2)).reshape(H, E * M),
    ], axis=1)
    wrz = np.concatenate([gru_W_hh[:2 * H].T, gru_W_ih[:2 * H].T], axis=0)
    shared = {
        "Wc": np.ascontiguousarray(wc).astype(bf),
        "Wrz": np.ascontiguousarray(wrz).astype(bf),
        # pre-halved: the device computes r*gh_n as (tanh+1) * (0.5*gh_n)
        "Wnh": np.ascontiguousarray(0.5 * gru_W_hh[2 * H:].T).astype(bf),
        "Wni": np.ascontiguousarray(gru_W_ih[2 * H:].T).astype(bf),
        "Wga": np.ascontiguousarray(gather_att_W).astype(bf),
        "Wge": np.ascontiguousarray(gather_emb_W).astype(bf),
    }
    in_maps = []
    for ci in range(NCORES):
        nsh = np.asarray(nodes[ci * G:(ci + 1) * G], np.float32)   # [G,V,NF]
        esh = np.asarray(edges[ci * G:(ci + 1) * G], np.float32)   # [G,V,V,E]
        nodesT = np.ascontiguousarray(
            nsh.transpose(2, 0, 1).reshape(NF, VG)).astype(bf)
        # block-diagonal edges^T: [pair, 128(w), E, 128(v)]; graph 2c+h's
        # edge matrix sits in rows/cols h*64:(h+1)*64, the rest is zero.
        et = esh.transpose(0, 2, 3, 1)                  # [G, w, e, v]
        edges_p = np.zeros((NPAIR, 128, E, 128), np.float32)
        edges_p[:, 0:64, :, 0:64] = et[0::2]
        edges_p[:, 64:128, :, 64:128] = et[1::2]
        in_maps.append({"nodesT": nodesT,
                        "edges_p": edges_p.astype(bf), **shared})
    return in_maps


def kernel(**inputs):
    global LAST_RESULTS
    from concourse.bass_utils import run_bass_kernel_spmd

    nc = build()
    in_maps = make_in_maps(**inputs)
    res = run_bass_kernel_spmd(nc, in_maps, core_ids=list(range(NCORES)),
                               trace=TRACE)
    LAST_RESULTS = res
    # red[d, j, g] -> out[g, j*128 + d] per core
    return np.concatenate(
        [np.asarray(r["red"], np.float32).transpose(2, 1, 0).reshape(G, OUT)
         for r in res.results], axis=0)
